# revision 13
# baseline (speedup 1.0000x reference)
"""Trainium2 Bass kernel for nn_NN_Dag_90967407329653 (dense_mlp).

Computation (per node n of D=128 independent nodes, batch B=4096):
    h1 = sigmoid(x @ W1_n.T + b1_n)        # 128 -> 256
    h2 = sigmoid(h1 @ Wa_n + ba_n)         # 256 -> 128
    out[:, n] = h2 @ Wb_n + bb_n           # 128 -> 1

Sharding: nodes across the 8 cores (16 nodes/core), full batch per core.
All activations kept transposed (features on partitions, batch on free dim)
so matmul weights are stationary and biases are per-partition (folded into
the sigmoid ACT instruction for free). Matmuls run in float32r (full PE
rate). Layer-3 row outputs are drained by DVE into a [16, B] output tile;
the host reassembles/transposes.
"""

import sys

sys.path.insert(0, "/opt/trn_rl_repo")

import numpy as np

import concourse.bass as bass
import concourse.tile as tile
from concourse import bacc, mybir
from concourse.bass_utils import run_bass_kernel_spmd

B = 4096  # batch
D = 128  # number of nodes
M1 = 256
M2 = 128
NCORES = 8
NPN = D // NCORES  # nodes per core = 16
W = 1024  # batch chunk width (2 PSUM banks)
NQ = B // W  # 4 chunks

F32 = mybir.dt.float32
F32R = mybir.dt.float32r
SIG = mybir.ActivationFunctionType.Sigmoid

_CACHE = {}


def _build(reps=1):
    nc = bacc.Bacc("TRN2", target_bir_lowering=False, debug=False)

    # weights packed [128, 4096 + 4096 + 16]: w1t | wa | wbt   (float32r)
    # biases packed [128, 32 + 16 + 16]:      b1t | bat | bb   (float32)
    WR_COLS = NPN * M1 + NPN * 2 * M2 + NPN
    BF_COLS = NPN * 2 + NPN + NPN
    xt_d = nc.declare_dram_parameter("xt", [D, B], F32R, isOutput=False)
    wr_d = nc.declare_dram_parameter("wr", [128, WR_COLS], F32R, isOutput=False)
    bf_d = nc.declare_dram_parameter("bf", [128, BF_COLS], F32, isOutput=False)
    out_d = nc.declare_dram_parameter("outt", [NPN, B], F32, isOutput=True)

    with tile.TileContext(nc) as tc:
        with (
            tc.tile_pool(name="const", bufs=1) as const,
            tc.tile_pool(name="act", bufs=2) as actp,
            tc.tile_pool(name="outp", bufs=4) as outp,
            tc.tile_pool(name="p1", bufs=2, space="PSUM") as p1,
            tc.tile_pool(name="p2", bufs=2, space="PSUM") as p2,
        ):
            xt = const.tile([D, B], F32R)
            wr = const.tile([128, WR_COLS], F32R)
            bfc = const.tile([128, BF_COLS], F32)
            nc.sync.dma_start(out=xt[:], in_=xt_d[:])
            nc.sync.dma_start(out=wr[:], in_=wr_d[:])
            nc.sync.dma_start(out=bfc[:], in_=bf_d[:])
            w1t = wr[:, 0 : NPN * M1]
            wa = wr[:, NPN * M1 : NPN * M1 + NPN * 2 * M2]
            wbt = wr[:, NPN * M1 + NPN * 2 * M2 :]
            b1t = bfc[:, 0 : NPN * 2]
            bat = bfc[:, NPN * 2 : NPN * 3]
            bb = bfc[:, NPN * 3 :]

            for _rep in range(reps):
              for j in range(NPN):
                for q in range(NQ):
                    # ---- layer 1: z1 = W1_n.T-chunk @ x, for both 128-wide
                    # output chunks; sigmoid+bias fused on ACT.
                    hs = []
                    for ofc in range(2):
                        z1 = p1.tile([128, W], F32, tag="z1")
                        lhs = w1t[:, j * M1 + ofc * 128 : j * M1 + (ofc + 1) * 128]
                        for s in range(W // 512):
                            nc.tensor.matmul(
                                z1[:, s * 512 : (s + 1) * 512],
                                lhsT=lhs,
                                rhs=xt[:, q * W + s * 512 : q * W + (s + 1) * 512],
                                start=True,
                                stop=True,
                            )
                        h1 = actp.tile([128, W], F32R, tag=f"h1{ofc}")
                        nc.scalar.activation(
                            h1[:],
                            z1[:],
                            SIG,
                            bias=b1t[:, 2 * j + ofc : 2 * j + ofc + 1],
                        )
                        hs.append(h1)

                    # ---- layer 2: z2 = sum_kc Wa_n[kc].T-as-lhsT @ h1[kc]
                    z2 = p2.tile([128, W], F32, tag="z2")
                    for s in range(W // 512):
                        sl = slice(s * 512, (s + 1) * 512)
                        for kc in range(2):
                            nc.tensor.matmul(
                                z2[:, sl],
                                lhsT=wa[
                                    :, (2 * j + kc) * M2 : (2 * j + kc + 1) * M2
                                ],
                                rhs=hs[kc][:, sl],
                                start=(kc == 0),
                                stop=(kc == 1),
                            )
                    h2 = actp.tile([128, W], F32R, tag="h2")
                    nc.scalar.activation(h2[:], z2[:], SIG, bias=bat[:, j : j + 1])

                    # ---- layer 3: out row = Wb_n.T @ h2 (+bb on the DVE
                    # drain). Reuses z2's PSUM banks after the sigmoid read.
                    for s in range(W // 512):
                        sl = slice(s * 512, (s + 1) * 512)
                        nc.tensor.matmul(
                            z2[0:1, sl],
                            lhsT=wbt[:, j : j + 1],
                            rhs=h2[:, sl],
                            start=True,
                            stop=True,
                        )
                    orow = outp.tile([1, W], F32, tag="orow")
                    nc.vector.tensor_scalar_add(
                        orow[0:1, 0:W],
                        z2[0:1, 0:W],
                        bb[0:1, j : j + 1],
                    )
                    nc.sync.dma_start(
                        out=out_d[j : j + 1, q * W : (q + 1) * W],
                        in_=orow[0:1, 0:W],
                    )

    nc.compile()
    return nc


def _in_maps(x, W1, b1, Wa, ba, Wb, bb):
    x = np.asarray(x, np.float32)
    W1 = np.asarray(W1, np.float32)
    b1 = np.asarray(b1, np.float32)
    Wa = np.asarray(Wa, np.float32)
    ba = np.asarray(ba, np.float32)
    Wb = np.asarray(Wb, np.float32)
    bb = np.asarray(bb, np.float32)

    xt = np.ascontiguousarray(x.T)  # [D, B]
    W1r = W1.reshape(D, M1, D)  # [n, m, k]
    b1r = b1.reshape(D, M1)
    maps = []
    for c in range(NCORES):
        nd = slice(c * NPN, (c + 1) * NPN)
        w1t = np.ascontiguousarray(
            W1r[nd].transpose(2, 0, 1).reshape(D, NPN * M1)
        )
        b1t = np.ascontiguousarray(
            b1r[nd].reshape(NPN, 2, 128).transpose(2, 0, 1).reshape(128, NPN * 2)
        )
        wa = np.ascontiguousarray(
            Wa[nd].reshape(NPN, 2, 128, M2).transpose(2, 0, 1, 3).reshape(128, -1)
        )
        bat = np.ascontiguousarray(ba[nd].T)
        wbt = np.ascontiguousarray(Wb[nd, :, 0].T)
        bbp = np.zeros((128, NPN), np.float32)
        bbp[0, :] = bb[nd, 0]
        wr = np.ascontiguousarray(np.concatenate([w1t, wa, wbt], axis=1))
        bf = np.ascontiguousarray(np.concatenate([b1t, bat, bbp], axis=1))
        maps.append(dict(xt=xt, wr=wr, bf=bf))
    return maps


def run(inputs, trace=False, reps=1):
    """Run on 8 cores; returns (out [B, D] fp32, BassKernelResults)."""
    key = ("nc", reps)
    if key not in _CACHE:
        _CACHE[key] = _build(reps)
    nc = _CACHE[key]
    maps = _in_maps(**inputs)
    res = run_bass_kernel_spmd(nc, maps, list(range(NCORES)), trace=trace)
    outt = np.concatenate([r["outt"] for r in res.results], axis=0)  # [D, B]
    return np.ascontiguousarray(outt.T), res


def kernel(**inputs):
    out, _ = run(inputs, trace=False)
    return out


# revision 20
# speedup vs baseline: 1.0606x; 1.0606x over previous
"""Trainium2 Bass kernel for nn_NN_Dag_90967407329653 (dense_mlp).

Computation (per node n of D=128 independent nodes, batch B=4096):
    h1 = sigmoid(x @ W1_n.T + b1_n)        # 128 -> 256
    h2 = sigmoid(h1 @ Wa_n + ba_n)         # 256 -> 128
    out[:, n] = h2 @ Wb_n + bb_n           # 128 -> 1

Sharding: nodes across the 8 cores (16 nodes/core), full batch per core.
All activations kept transposed (features on partitions, batch on free dim)
so matmul weights are stationary and biases are per-partition (folded into
the sigmoid ACT instruction for free). Matmuls run in float32r (full PE
rate). Layer-3 row outputs are drained by DVE into a [16, B] output tile;
the host reassembles/transposes.
"""

import sys

sys.path.insert(0, "/opt/trn_rl_repo")

import numpy as np

import concourse.bass as bass
import concourse.tile as tile
from concourse import bacc, mybir
from concourse.bass_utils import run_bass_kernel_spmd

B = 4096  # batch
D = 128  # number of nodes
M1 = 256
M2 = 128
NCORES = 8
NPN = D // NCORES  # nodes per core = 16
W = 1024  # batch chunk width (2 PSUM banks)
NQ = B // W  # 4 chunks

F32 = mybir.dt.float32
F32R = mybir.dt.float32r
SIG = mybir.ActivationFunctionType.Sigmoid

_CACHE = {}


def _build(reps=1):
    nc = bacc.Bacc("TRN2", target_bir_lowering=False, debug=False)

    # weights packed [128, 4096 + 4096 + 16]: w1t | wa | wbt   (float32r)
    # biases packed [128, 32 + 16 + 16]:      b1t | bat | bb   (float32)
    WR_COLS = NPN * M1 + NPN * 2 * M2 + NPN
    BF_COLS = NPN * 2 + NPN + NPN
    xt_d = nc.declare_dram_parameter("xt", [D, B], F32R, isOutput=False)
    wr_d = nc.declare_dram_parameter("wr", [128, WR_COLS], F32R, isOutput=False)
    bf_d = nc.declare_dram_parameter("bf", [128, BF_COLS], F32, isOutput=False)
    out_d = nc.declare_dram_parameter("outt", [NPN, B], F32, isOutput=True)

    with tile.TileContext(nc) as tc:
        with (
            tc.tile_pool(name="const", bufs=1) as const,
            tc.tile_pool(name="act", bufs=3) as actp,
            tc.tile_pool(name="outp", bufs=8) as outp,
            tc.tile_pool(name="p1", bufs=2, space="PSUM") as p1,
            tc.tile_pool(name="p2", bufs=2, space="PSUM") as p2,
        ):
            xt = const.tile([D, B], F32R)
            wr = const.tile([128, WR_COLS], F32R)
            bfc = const.tile([128, BF_COLS], F32)
            # Chunked loads: range-based dep tracking lets the first
            # matmuls start as soon as their slice has landed.
            nc.sync.dma_start(out=bfc[:], in_=bf_d[:])
            nc.sync.dma_start(out=wr[:, 0:512], in_=wr_d[:, 0:512])
            for c in range(8):
                s = slice(c * (B // 8), (c + 1) * (B // 8))
                nc.sync.dma_start(out=xt[:, s], in_=xt_d[:, s])
            wq = (WR_COLS - 512) // 4
            for c in range(4):
                s = slice(512 + c * wq, 512 + (c + 1) * wq)
                nc.sync.dma_start(out=wr[:, s], in_=wr_d[:, s])
            w1t = wr[:, 0 : NPN * M1]
            wa = wr[:, NPN * M1 : NPN * M1 + NPN * 2 * M2]
            wbt = wr[:, NPN * M1 + NPN * 2 * M2 :]
            b1t = bfc[:, 0 : NPN * 2]
            bat = bfc[:, NPN * 2 : NPN * 3]
            bb = bfc[:, NPN * 3 :]

            for _rep in range(reps):
              for j in range(NPN):
                for q in range(NQ):
                    # ---- layer 1: z1 = W1_n.T-chunk @ x, for both 128-wide
                    # output chunks; sigmoid+bias fused on ACT.
                    hs = []
                    for ofc in range(2):
                        z1 = p1.tile([128, W], F32, tag="z1")
                        lhs = w1t[:, j * M1 + ofc * 128 : j * M1 + (ofc + 1) * 128]
                        for s in range(W // 512):
                            nc.tensor.matmul(
                                z1[:, s * 512 : (s + 1) * 512],
                                lhsT=lhs,
                                rhs=xt[:, q * W + s * 512 : q * W + (s + 1) * 512],
                                start=True,
                                stop=True,
                            )
                        h1 = actp.tile([128, W], F32R, tag=f"h1{ofc}")
                        nc.scalar.activation(
                            h1[:],
                            z1[:],
                            SIG,
                            bias=b1t[:, 2 * j + ofc : 2 * j + ofc + 1],
                        )
                        hs.append(h1)

                    # ---- layer 2: z2 = sum_kc Wa_n[kc].T-as-lhsT @ h1[kc]
                    z2 = p2.tile([128, W], F32, tag="z2")
                    for s in range(W // 512):
                        sl = slice(s * 512, (s + 1) * 512)
                        for kc in range(2):
                            nc.tensor.matmul(
                                z2[:, sl],
                                lhsT=wa[
                                    :, (2 * j + kc) * M2 : (2 * j + kc + 1) * M2
                                ],
                                rhs=hs[kc][:, sl],
                                start=(kc == 0),
                                stop=(kc == 1),
                            )
                    h2 = actp.tile([128, W], F32R, tag="h2")
                    nc.scalar.activation(h2[:], z2[:], SIG, bias=bat[:, j : j + 1])

                    # ---- layer 3: out row = Wb_n.T @ h2 (+bb on the DVE
                    # drain). Reuses z2's PSUM banks after the sigmoid read.
                    for s in range(W // 512):
                        sl = slice(s * 512, (s + 1) * 512)
                        nc.tensor.matmul(
                            z2[0:1, sl],
                            lhsT=wbt[:, j : j + 1],
                            rhs=h2[:, sl],
                            start=True,
                            stop=True,
                        )
                    orow = outp.tile([1, W], F32, tag="orow")
                    nc.vector.tensor_scalar_add(
                        orow[0:1, 0:W],
                        z2[0:1, 0:W],
                        bb[0:1, j : j + 1],
                    )
                    nc.sync.dma_start(
                        out=out_d[j : j + 1, q * W : (q + 1) * W],
                        in_=orow[0:1, 0:W],
                    )

    nc.compile()
    return nc


def _in_maps(x, W1, b1, Wa, ba, Wb, bb):
    x = np.asarray(x, np.float32)
    W1 = np.asarray(W1, np.float32)
    b1 = np.asarray(b1, np.float32)
    Wa = np.asarray(Wa, np.float32)
    ba = np.asarray(ba, np.float32)
    Wb = np.asarray(Wb, np.float32)
    bb = np.asarray(bb, np.float32)

    xt = np.ascontiguousarray(x.T)  # [D, B]
    W1r = W1.reshape(D, M1, D)  # [n, m, k]
    b1r = b1.reshape(D, M1)
    maps = []
    for c in range(NCORES):
        nd = slice(c * NPN, (c + 1) * NPN)
        w1t = np.ascontiguousarray(
            W1r[nd].transpose(2, 0, 1).reshape(D, NPN * M1)
        )
        b1t = np.ascontiguousarray(
            b1r[nd].reshape(NPN, 2, 128).transpose(2, 0, 1).reshape(128, NPN * 2)
        )
        wa = np.ascontiguousarray(
            Wa[nd].reshape(NPN, 2, 128, M2).transpose(2, 0, 1, 3).reshape(128, -1)
        )
        bat = np.ascontiguousarray(ba[nd].T)
        wbt = np.ascontiguousarray(Wb[nd, :, 0].T)
        bbp = np.zeros((128, NPN), np.float32)
        bbp[0, :] = bb[nd, 0]
        wr = np.ascontiguousarray(np.concatenate([w1t, wa, wbt], axis=1))
        bf = np.ascontiguousarray(np.concatenate([b1t, bat, bbp], axis=1))
        maps.append(dict(xt=xt, wr=wr, bf=bf))
    return maps


def run(inputs, trace=False, reps=1):
    """Run on 8 cores; returns (out [B, D] fp32, BassKernelResults)."""
    key = ("nc", reps)
    if key not in _CACHE:
        _CACHE[key] = _build(reps)
    nc = _CACHE[key]
    maps = _in_maps(**inputs)
    res = run_bass_kernel_spmd(nc, maps, list(range(NCORES)), trace=trace)
    outt = np.concatenate([r["outt"] for r in res.results], axis=0)  # [D, B]
    return np.ascontiguousarray(outt.T), res


def kernel(**inputs):
    out, _ = run(inputs, trace=False)
    return out


# revision 24
# speedup vs baseline: 1.0646x; 1.0038x over previous
"""Trainium2 Bass kernel for nn_NN_Dag_90967407329653 (dense_mlp).

Computation (per node n of D=128 independent nodes, batch B=4096):
    h1 = sigmoid(x @ W1_n.T + b1_n)        # 128 -> 256
    h2 = sigmoid(h1 @ Wa_n + ba_n)         # 256 -> 128
    out[:, n] = h2 @ Wb_n + bb_n           # 128 -> 1

Sharding: nodes across the 8 cores (16 nodes/core), full batch per core.
All activations kept transposed (features on partitions, batch on free dim)
so matmul weights are stationary and biases are per-partition (folded into
the sigmoid ACT instruction for free). Matmuls run in float32r (full PE
rate). Layer-3 row outputs are drained by DVE into a [16, B] output tile;
the host reassembles/transposes.
"""

import sys

sys.path.insert(0, "/opt/trn_rl_repo")

import numpy as np

import concourse.bass as bass
import concourse.tile as tile
from concourse import bacc, mybir
from concourse.bass_utils import run_bass_kernel_spmd

B = 4096  # batch
D = 128  # number of nodes
M1 = 256
M2 = 128
NCORES = 8
NPN = D // NCORES  # nodes per core = 16
W = 1024  # batch chunk width (2 PSUM banks)
NQ = B // W  # 4 chunks

F32 = mybir.dt.float32
F32R = mybir.dt.float32r
SIG = mybir.ActivationFunctionType.Sigmoid

_CACHE = {}


def _build(reps=1):
    nc = bacc.Bacc("TRN2", target_bir_lowering=False, debug=False)

    # weights packed [128, 4096 + 4096 + 16]: w1t | wa | wbt   (float32r)
    # biases packed [128, 32 + 16 + 16]:      b1t | bat | bb   (float32)
    WR_COLS = NPN * M1 + NPN * 2 * M2 + NPN
    BF_COLS = NPN * 2 + NPN + NPN
    xt_d = nc.declare_dram_parameter("xt", [D, B], F32R, isOutput=False)
    wr_d = nc.declare_dram_parameter("wr", [128, WR_COLS], F32R, isOutput=False)
    bf_d = nc.declare_dram_parameter("bf", [128, BF_COLS], F32, isOutput=False)
    out_d = nc.declare_dram_parameter("outt", [NPN, B], F32, isOutput=True)

    with tile.TileContext(nc) as tc:
        with (
            tc.tile_pool(name="const", bufs=1) as const,
            tc.tile_pool(name="act", bufs=4) as actp,
            tc.tile_pool(name="outp", bufs=8) as outp,
            tc.tile_pool(name="p1", bufs=2, space="PSUM") as p1,
            tc.tile_pool(name="p2", bufs=2, space="PSUM") as p2,
        ):
            xt = const.tile([D, B], F32R)
            wr = const.tile([128, WR_COLS], F32R)
            bfc = const.tile([128, BF_COLS], F32)
            # Chunked loads: range-based dep tracking lets the first
            # matmuls start as soon as their slice has landed.
            nc.sync.dma_start(out=bfc[:], in_=bf_d[:])
            nc.sync.dma_start(out=wr[:, 0:512], in_=wr_d[:, 0:512])
            for c in range(8):
                s = slice(c * (B // 8), (c + 1) * (B // 8))
                nc.sync.dma_start(out=xt[:, s], in_=xt_d[:, s])
            wq = (WR_COLS - 512) // 4
            for c in range(4):
                s = slice(512 + c * wq, 512 + (c + 1) * wq)
                nc.sync.dma_start(out=wr[:, s], in_=wr_d[:, s])

            # Warm the sigmoid ACT table (~2.7us load) during the input
            # DMAs instead of on the first real sigmoid.
            warm = const.tile([1, 1], F32)
            nc.vector.memset(warm[:], 0.0)
            nc.scalar.activation(warm[:], warm[:], SIG, bias=0.0)
            w1t = wr[:, 0 : NPN * M1]
            wa = wr[:, NPN * M1 : NPN * M1 + NPN * 2 * M2]
            wbt = wr[:, NPN * M1 + NPN * 2 * M2 :]
            b1t = bfc[:, 0 : NPN * 2]
            bat = bfc[:, NPN * 2 : NPN * 3]
            bb = bfc[:, NPN * 3 :]

            for _rep in range(reps):
              for j in range(NPN):
                for q in range(NQ):
                    # ---- layer 1: z1 = W1_n.T-chunk @ x, for both 128-wide
                    # output chunks; sigmoid+bias fused on ACT.
                    hs = []
                    for ofc in range(2):
                        z1 = p1.tile([128, W], F32, tag="z1")
                        lhs = w1t[:, j * M1 + ofc * 128 : j * M1 + (ofc + 1) * 128]
                        for s in range(W // 512):
                            nc.tensor.matmul(
                                z1[:, s * 512 : (s + 1) * 512],
                                lhsT=lhs,
                                rhs=xt[:, q * W + s * 512 : q * W + (s + 1) * 512],
                                start=True,
                                stop=True,
                            )
                        h1 = actp.tile([128, W], F32R, tag=f"h1{ofc}")
                        nc.scalar.activation(
                            h1[:],
                            z1[:],
                            SIG,
                            bias=b1t[:, 2 * j + ofc : 2 * j + ofc + 1],
                        )
                        hs.append(h1)

                    # ---- layer 2: z2 = sum_kc Wa_n[kc].T-as-lhsT @ h1[kc]
                    z2 = p2.tile([128, W], F32, tag="z2")
                    for s in range(W // 512):
                        sl = slice(s * 512, (s + 1) * 512)
                        for kc in range(2):
                            nc.tensor.matmul(
                                z2[:, sl],
                                lhsT=wa[
                                    :, (2 * j + kc) * M2 : (2 * j + kc + 1) * M2
                                ],
                                rhs=hs[kc][:, sl],
                                start=(kc == 0),
                                stop=(kc == 1),
                            )
                    h2 = actp.tile([128, W], F32R, tag="h2")
                    nc.scalar.activation(h2[:], z2[:], SIG, bias=bat[:, j : j + 1])

                    # ---- layer 3: out row = Wb_n.T @ h2 (+bb on the DVE
                    # drain). Reuses z2's PSUM banks after the sigmoid read.
                    for s in range(W // 512):
                        sl = slice(s * 512, (s + 1) * 512)
                        nc.tensor.matmul(
                            z2[0:1, sl],
                            lhsT=wbt[:, j : j + 1],
                            rhs=h2[:, sl],
                            start=True,
                            stop=True,
                        )
                    orow = outp.tile([1, W], F32, tag="orow")
                    nc.vector.tensor_scalar_add(
                        orow[0:1, 0:W],
                        z2[0:1, 0:W],
                        bb[0:1, j : j + 1],
                    )
                    nc.sync.dma_start(
                        out=out_d[j : j + 1, q * W : (q + 1) * W],
                        in_=orow[0:1, 0:W],
                    )

    nc.compile()
    return nc


def _in_maps(x, W1, b1, Wa, ba, Wb, bb):
    x = np.asarray(x, np.float32)
    W1 = np.asarray(W1, np.float32)
    b1 = np.asarray(b1, np.float32)
    Wa = np.asarray(Wa, np.float32)
    ba = np.asarray(ba, np.float32)
    Wb = np.asarray(Wb, np.float32)
    bb = np.asarray(bb, np.float32)

    xt = np.ascontiguousarray(x.T)  # [D, B]
    W1r = W1.reshape(D, M1, D)  # [n, m, k]
    b1r = b1.reshape(D, M1)
    maps = []
    for c in range(NCORES):
        nd = slice(c * NPN, (c + 1) * NPN)
        w1t = np.ascontiguousarray(
            W1r[nd].transpose(2, 0, 1).reshape(D, NPN * M1)
        )
        b1t = np.ascontiguousarray(
            b1r[nd].reshape(NPN, 2, 128).transpose(2, 0, 1).reshape(128, NPN * 2)
        )
        wa = np.ascontiguousarray(
            Wa[nd].reshape(NPN, 2, 128, M2).transpose(2, 0, 1, 3).reshape(128, -1)
        )
        bat = np.ascontiguousarray(ba[nd].T)
        wbt = np.ascontiguousarray(Wb[nd, :, 0].T)
        bbp = np.zeros((128, NPN), np.float32)
        bbp[0, :] = bb[nd, 0]
        wr = np.ascontiguousarray(np.concatenate([w1t, wa, wbt], axis=1))
        bf = np.ascontiguousarray(np.concatenate([b1t, bat, bbp], axis=1))
        maps.append(dict(xt=xt, wr=wr, bf=bf))
    return maps


def run(inputs, trace=False, reps=1):
    """Run on 8 cores; returns (out [B, D] fp32, BassKernelResults)."""
    key = ("nc", reps)
    if key not in _CACHE:
        _CACHE[key] = _build(reps)
    nc = _CACHE[key]
    maps = _in_maps(**inputs)
    res = run_bass_kernel_spmd(nc, maps, list(range(NCORES)), trace=trace)
    outt = np.concatenate([r["outt"] for r in res.results], axis=0)  # [D, B]
    return np.ascontiguousarray(outt.T), res


def kernel(**inputs):
    out, _ = run(inputs, trace=False)
    return out
